# revision 16
# baseline (speedup 1.0000x reference)
"""MoE gating kernel for Trainium2 (8 NeuronCores, token-parallel).

Computes, for x:[4,4096,4096] f32, W:[64,4096] f32, b:[64] f32:
  logits = x @ W.T + b                      # [B,S,64]
  top-8 per token, sparse softmax over the top-8 positions
Returns (sparse_logits [B,S,64] f32, indices [B,S,8] i32, gate_logit [B*S,64] f32)

Sharding: tokens (B*S = 16384) split evenly across 8 cores (2048 tokens/core);
W and b replicated. No cross-core communication.

Per-core pipeline:
  - DMA x rows in naturally as [128 tok, 4096] tiles (contiguous, fast).
  - PE transpose 128x128 blocks to put H on partitions (PSUM -> SBUF via
    ACT/DVE copies, split evenly).
  - Matmul with the xT block STATIONARY and the 64-col W^T chunk MOVING:
    fp32 matmul cost scales with moving columns (4 cyc/col), so moving 64
    cols instead of 512 halves PE stream time, and the output lands
    directly in [token, expert] layout (no back-transposes needed).
  - Bias added on DVE during PSUM eviction from a partition-replicated b.
  - DVE max/max_index give top-8 values + indices per token; sparse
    softmax via exp(l - m0 - ln Z) with Z = sum of top-8 exps; mask with
    l >= m7 (8th max).
"""

import os
import sys

import numpy as np

if "/opt/trn_rl_repo" not in sys.path:
    sys.path.insert(0, "/opt/trn_rl_repo")

N_CORES = 8
TOKENS = 16384
TOK_PC = TOKENS // N_CORES  # 2048
H = 4096
E = 64
K = 8
TCH = 512                    # tokens per chunk
N_TCH = TOK_PC // TCH        # 4
N_SUB = TCH // 128           # 4
N_KCH = H // 128             # 32

_PROGRAM = None


def _build_program(repeat: int = 1):
    import concourse.bacc as bacc
    import concourse.mybir as mybir
    import concourse.tile as tile
    from concourse.masks import make_identity
    from contextlib import ExitStack

    f32 = mybir.dt.float32
    AF = mybir.ActivationFunctionType
    ALU = mybir.AluOpType
    AX = mybir.AxisListType

    nc = bacc.Bacc(
        "TRN2",
        target_bir_lowering=False,
        debug=False,
        enable_asserts=False,
        num_devices=N_CORES,
    )

    x_in = nc.dram_tensor("x", [TOK_PC, H], f32, kind="ExternalInput").ap()
    w_in = nc.dram_tensor("W", [E, H], f32, kind="ExternalInput").ap()
    b_in = nc.dram_tensor("b", [E], f32, kind="ExternalInput").ap()
    sparse_out = nc.dram_tensor("sparse", [TOK_PC, E], f32, kind="ExternalOutput").ap()
    idx_out = nc.dram_tensor(
        "indices", [TOK_PC, K], mybir.dt.int32, kind="ExternalOutput"
    ).ap()
    gate_out = nc.dram_tensor("gate", [TOK_PC, E], f32, kind="ExternalOutput").ap()

    with tile.TileContext(nc) as tc, ExitStack() as ctx:
        const = ctx.enter_context(tc.tile_pool(name="const", bufs=1))

        ident = const.tile([128, 128], f32)
        make_identity(nc, ident)

        w_sb = const.tile([E, H], f32)
        nc.sync.dma_start(out=w_sb, in_=w_in)
        # b replicated across all 128 partitions (partition-step-0 DMA)
        import concourse.bass as bass

        b_rep = const.tile([128, E], f32)
        nc.gpsimd.dma_start(
            out=b_rep,
            in_=bass.AP(
                tensor=b_in.tensor, offset=b_in.offset, ap=[[0, 128]] + list(b_in.ap)
            ),
        )

        # W^T, chunked over H: wt_sb[:, k, :] is [128 h, 64 e] for h-chunk k.
        wt_sb = const.tile([128, N_KCH, E], f32)
        with tc.tile_pool(name="wtps", bufs=2, space="PSUM") as wtps:
            for kk in range(N_KCH):
                wt_ps = wtps.tile([128, E], f32, name=f"wt_ps_{kk}", tag="wt_ps")
                nc.tensor.transpose(
                    wt_ps, w_sb[:, kk * 128 : (kk + 1) * 128], ident[:E, :E]
                )
                nc.scalar.copy(wt_sb[:, kk, :], wt_ps)

        xpool = ctx.enter_context(tc.tile_pool(name="xpool", bufs=16))
        xtps = ctx.enter_context(tc.tile_pool(name="xtps", bufs=4, space="PSUM"))
        xtsb = ctx.enter_context(tc.tile_pool(name="xtsb", bufs=4))
        tokps = ctx.enter_context(tc.tile_pool(name="tokps", bufs=3, space="PSUM"))
        work = ctx.enter_context(tc.tile_pool(name="work", bufs=3))
        outp = ctx.enter_context(tc.tile_pool(name="outp", bufs=3))

        KQ = 4  # k-chunks transposed per PSUM bank fill
        for t in range(N_TCH * repeat):
            rep, t = divmod(t, N_TCH)
            xs = []
            for s in range(N_SUB):
                r0 = t * TCH + s * 128
                halves = []
                for h in range(2):
                    x_t = xpool.tile(
                        [128, H // 2], f32, name=f"x_{t}_{s}_{h}", tag="x"
                    )
                    nc.sync.dma_start(
                        out=x_t,
                        in_=x_in[r0 : r0 + 128, h * (H // 2) : (h + 1) * (H // 2)],
                    )
                    halves.append(x_t)
                xs.append(halves)

            logits = outp.tile([128, N_SUB, E], f32, name=f"logits_{t}", tag="logits")
            # per 128-token sub-block: accumulate logits [128 tok, 64 e] in
            # PSUM with the xT blocks stationary and the 64-col W^T chunk
            # moving (fp32 matmul cost scales with moving cols: 64 not 512).
            # One accumulation group open at a time; transposes for the next
            # k-quad interleave with the current group's matmuls.
            for s in range(N_SUB):
                tok_ps = tokps.tile([128, E], f32, name=f"tok_ps_{t}_{s}", tag="tok_ps")
                for kq in range(N_KCH // KQ):
                    xT_ps = xtps.tile(
                        [128, KQ * 128], f32, name=f"xT_ps_{t}_{s}_{kq}", tag="xT_ps"
                    )
                    for j in range(KQ):
                        kk = kq * KQ + j
                        half, kh = divmod(kk, N_KCH // 2)
                        nc.tensor.transpose(
                            xT_ps[:, j * 128 : (j + 1) * 128],
                            xs[s][half][:, kh * 128 : (kh + 1) * 128],
                            ident,
                        )
                    xT_sb = xtsb.tile(
                        [128, KQ * 128], f32, name=f"xT_sb_{t}_{s}_{kq}", tag="xT_sb"
                    )
                    if kq % 2 == 1:
                        nc.vector.tensor_copy(xT_sb, xT_ps)
                    else:
                        nc.scalar.copy(xT_sb, xT_ps)
                    for j in range(KQ):
                        kk = kq * KQ + j
                        nc.tensor.matmul(
                            tok_ps,
                            lhsT=xT_sb[:, j * 128 : (j + 1) * 128],
                            rhs=wt_sb[:, kk, :],
                            start=(kk == 0),
                            stop=(kk == N_KCH - 1),
                        )
                # evict + bias (b replicated across partitions) on DVE
                nc.vector.tensor_add(logits[:, s, :], tok_ps, b_rep)

            # ---- top-8 + sparse softmax ----
            mx8 = work.tile([128, N_SUB, K], f32, name=f"mx8_{t}", tag="mx8")
            idx8 = work.tile(
                [128, N_SUB, K], mybir.dt.uint32, name=f"idx8_{t}", tag="idx8"
            )
            for s in range(N_SUB):
                nc.vector.max(out=mx8[:, s, :], in_=logits[:, s, :])
                nc.vector.max_index(
                    out=idx8[:, s, :], in_max=mx8[:, s, :], in_values=logits[:, s, :]
                )

            negm0 = work.tile([128, N_SUB], f32, name=f"negm0_{t}", tag="negm0")
            nc.vector.tensor_scalar_mul(negm0, mx8[:, :, 0], -1.0)

            # exp of top-8 relative to m0, summed -> Z, then bias2 = -m0 - ln Z
            mx8c = work.tile([128, N_SUB, K], f32, name=f"mx8c_{t}", tag="mx8c")
            nc.vector.tensor_tensor(
                out=mx8c,
                in0=mx8,
                in1=negm0.rearrange("p s -> p s ()").to_broadcast([128, N_SUB, K]),
                op=ALU.add,
            )
            e8 = work.tile([128, N_SUB, K], f32, name=f"e8_{t}", tag="e8")
            nc.scalar.activation(e8, mx8c, AF.Exp)
            z4 = work.tile([128, N_SUB], f32, name=f"z4_{t}", tag="z4")
            nc.vector.reduce_sum(z4, e8, axis=AX.X)
            lnz = work.tile([128, N_SUB], f32, name=f"lnz_{t}", tag="lnz")
            nc.scalar.activation(lnz, z4, AF.Ln)
            bias2 = work.tile([128, N_SUB], f32, name=f"bias2_{t}", tag="bias2")
            nc.vector.tensor_sub(bias2, negm0, lnz)

            # p = exp(l - m0 - lnZ); mask = l >= m7; sparse = mask * p
            lc = work.tile([128, N_SUB, E], f32, name=f"lc_{t}", tag="lc")
            nc.vector.tensor_tensor(
                out=lc,
                in0=logits,
                in1=bias2.rearrange("p s -> p s ()").to_broadcast([128, N_SUB, E]),
                op=ALU.add,
            )
            p_all = work.tile([128, N_SUB, E], f32, name=f"p_all_{t}", tag="p_all")
            nc.scalar.activation(p_all, lc, AF.Exp)
            mask = work.tile([128, N_SUB, E], f32, name=f"mask_{t}", tag="mask")
            nc.vector.tensor_tensor(
                out=mask,
                in0=logits,
                in1=mx8[:, :, 7].rearrange("p s -> p s ()").to_broadcast(
                    [128, N_SUB, E]
                ),
                op=ALU.is_ge,
            )
            sparse = outp.tile([128, N_SUB, E], f32, name=f"sparse_{t}", tag="sparse")
            nc.vector.tensor_mul(sparse, mask, p_all)

            # ---- DMA out (token = s*128 + p) ----
            rows = slice(t * TCH, (t + 1) * TCH)
            nc.sync.dma_start(
                out=gate_out[rows, :].rearrange("(s p) e -> p s e", p=128),
                in_=logits,
            )
            nc.sync.dma_start(
                out=sparse_out[rows, :].rearrange("(s p) e -> p s e", p=128),
                in_=sparse,
            )
            nc.sync.dma_start(
                out=idx_out[rows, :].rearrange("(s p) k -> p s k", p=128),
                in_=idx8.bitcast(mybir.dt.int32),
            )

    nc.compile()
    return nc


def _get_program():
    global _PROGRAM
    if _PROGRAM is None:
        _PROGRAM = _build_program()
    return _PROGRAM


LAST_RESULTS = None


def kernel(x: np.ndarray, W: np.ndarray, b: np.ndarray):
    global LAST_RESULTS
    from concourse.bass_utils import run_bass_kernel_spmd

    nc = _get_program()

    B, S, Hx = x.shape
    xf = np.ascontiguousarray(np.asarray(x, dtype=np.float32).reshape(B * S, Hx))
    Wf = np.ascontiguousarray(np.asarray(W, dtype=np.float32))
    bf = np.ascontiguousarray(np.asarray(b, dtype=np.float32))

    in_maps = [
        {"x": xf[i * TOK_PC : (i + 1) * TOK_PC], "W": Wf, "b": bf}
        for i in range(N_CORES)
    ]
    res = run_bass_kernel_spmd(
        nc,
        in_maps,
        list(range(N_CORES)),
        trace=bool(int(os.environ.get("KERNEL_TRACE", "0"))),
    )
    LAST_RESULTS = res
    outs = res.results

    sparse = np.concatenate([outs[i]["sparse"] for i in range(N_CORES)], axis=0)
    idx = np.concatenate([outs[i]["indices"] for i in range(N_CORES)], axis=0)
    gate = np.concatenate([outs[i]["gate"] for i in range(N_CORES)], axis=0)

    return (
        sparse.reshape(B, S, E),
        idx.reshape(B, S, K).astype(np.int32, copy=False),
        gate.reshape(B * S, E),
    )
